# revision 19
# baseline (speedup 1.0000x reference)
"""Trainium2 Bass kernel for ExpanderLinear: out = x @ (W * mask).T

Shapes (hardcoded): x [8192, 4096] f32, weight [4096, 4096] f32,
mask [4096, 4096] f32 -> out [8192, 4096] f32.

Strategy: tensor-parallel over output features across 8 cores, bf16,
with one level of Strassen so the PE does 7/8 of the dense matmul work.
Per core: outT_c = A @ B where A = (W*mask)[c-slice] [512, 4096] and
B = x.T [4096, 8192]. Split A 2x2 (outs 256, K 2048), B 2x2 (K 2048,
batch 4096). The 7 A-combos are built on the host (free) and shipped as
bf16 lhsT panels; the 5 B-combos are built on the idle Vector engine
from streamed x panels; products accumulate in PSUM and are assembled
into C quadrants by DVE adds, then DMA'd out.

Batch is processed as 8 chunk-pairs (bc, bc+8) of 512 columns from each
batch half. Per pair: 7 products x 2 psum tiles x 16 K-chunk matmuls
= 224 MMs (dense would be 256).

Host pre-tiles everything contiguously:
  xt [16 bc][8 g][128 p][4 kc][512 b] bf16 (panel = 512 KB)
  wa [7 s][128 p][16 ic][256 o]       bf16 (A-combo lhsT panels)
  outT [4 ot][16 bc][128 p][512 b]    f32  (host re-assembles)
"""

import ml_dtypes
import numpy as np

import concourse.bass as bass
import concourse.mybir as mybir
import concourse.tile as tile
from concourse import bacc
from concourse.bass_utils import run_bass_kernel_spmd

P = 128
D_IN = 4096
D_OUT = 4096
BATCH = 8192
N_CORES = 8
O_PER_CORE = D_OUT // N_CORES  # 512
B_CHUNK = 512
N_BCHUNK = BATCH // B_CHUNK  # 16
N_PAIR = N_BCHUNK // 2  # 8
KG = 8  # 512-row K groups (4 per K-half)
KCG = 4  # 128-row chunks per group
KH_IC = 16  # 128-row chunks per K-half
OT = 4  # 128-row output tiles per core

F32 = mybir.dt.float32
BF16 = mybir.dt.bfloat16

# Products in issue order: (wa_idx, rhs_spec, contribs)
# rhs_spec: ('plain', side, gbase) or ('combo', op, (sideA,gbaseA), (sideB,gbaseB))
# combo sub g' = panel[sideA][gbaseA+g'] op panel[sideB][gbaseB+g']
# contribs: list of (row_base, side, sign); applied to C[row_base+o'][side]
# A-combos (host order): 0:A11+A22 1:A21+A22 2:A11 3:A22 4:A11+A12
#                        5:A21-A11 6:A12-A22
PRODUCTS = [
    # M2 = (A21+A22) @ B11
    (1, ("plain", "lo", 0), [(2, "lo", +1), (2, "hi", -1)]),
    # M5 = (A11+A12) @ B22
    (4, ("plain", "hi", 4), [(0, "lo", -1), (0, "hi", +1)]),
    # M1 = (A11+A22) @ (B11+B22)
    (0, ("combo", "add", ("lo", 0), ("hi", 4)), [(0, "lo", +1), (2, "hi", +1)]),
    # M4 = A22 @ (B21-B11)
    (3, ("combo", "sub", ("lo", 4), ("lo", 0)), [(0, "lo", +1), (2, "lo", +1)]),
    # M7 = (A12-A22) @ (B21+B22)
    (6, ("combo", "add", ("lo", 4), ("hi", 4)), [(0, "lo", +1)]),
    # M3 = A11 @ (B12-B22)
    (2, ("combo", "sub", ("hi", 0), ("hi", 4)), [(0, "hi", +1), (2, "hi", +1)]),
    # M6 = (A21-A11) @ (B11+B12)
    (5, ("combo", "add", ("lo", 0), ("hi", 0)), [(2, "hi", +1)]),
]

# Total contribs per C tile (row_base, side) per pair
_C_COUNTS = {}
for _, _, contribs in PRODUCTS:
    for rb, side, _sg in contribs:
        _C_COUNTS[(rb, side)] = _C_COUNTS.get((rb, side), 0) + 1

# panel arrival order for each pair (matches product consumption order)
PANEL_ORDER = (
    [("lo", g) for g in range(4)]
    + [("hi", g) for g in range(4, 8)]
    + [("lo", g) for g in range(4, 8)]
    + [("hi", g) for g in range(4)]
)
# wa arrival order (first-needed first)
WA_ORDER = [1, 4, 0, 3, 6, 2, 5]


def build_nc():
    nc = bacc.Bacc("TRN2", target_bir_lowering=False, debug=False, num_devices=N_CORES)

    xt_d = nc.dram_tensor("xt", [N_BCHUNK, KG, P, KCG, B_CHUNK], BF16, kind="ExternalInput")
    wa_d = nc.dram_tensor("wa", [7, P, KH_IC, 2 * P], BF16, kind="ExternalInput")
    out_d = nc.dram_tensor("outT", [OT, N_BCHUNK, P, B_CHUNK], F32, kind="ExternalOutput")

    with tile.TileContext(nc) as tc:
        with (
            tc.tile_pool(name="wa", bufs=1) as wapool,
            tc.tile_pool(name="xp", bufs=18) as xppool,
            tc.tile_pool(name="combo", bufs=6) as combopool,
            tc.tile_pool(name="cacc", bufs=8) as caccpool,
            tc.tile_pool(name="mpsum", bufs=8, space="PSUM") as mpsum,
        ):
            wa_t = [None] * 7

            def emit_wa(s):
                w = wapool.tile([P, KH_IC, 2 * P], BF16, name=f"wa{s}")
                nc.sync.dma_start(w, wa_d[s])
                wa_t[s] = w

            def emit_panel(bc, g):
                xs = xppool.tile([P, KCG, B_CHUNK], BF16, tag="xs", name="xs")
                nc.sync.dma_start(xs, xt_d[bc, g])
                return xs

            # --- prologue ---
            panels = {}  # (side, g) -> tile, for current pair

            def emit_pair0_panel():
                side, g = PANEL_ORDER[len(panels)]
                bc = 0 if side == "lo" else N_PAIR
                panels[(side, g)] = emit_panel(bc, g)

            # wa1 (the first product's lhsT) split into quarters with the
            # first x panel issued after q0, so the very first matmul waits
            # on 768 KB instead of 1.5 MB. Everything else keeps the v3
            # supply order, which measured best.
            wa1_subs = []
            for q in range(4):
                w = wapool.tile([P, KCG, 2 * P], BF16, name=f"wa1q{q}")
                nc.sync.dma_start(w, wa_d[1, :, 4 * q : 4 * q + 4, :])
                wa1_subs.append(w)
                if q == 0:
                    emit_pair0_panel()  # (lo, 0)
            wa_t[1] = wa1_subs
            for i, s in enumerate(WA_ORDER[1:]):
                emit_wa(s)
                for _ in range(3 if i < 1 else 2):
                    if len(panels) < 16:
                        emit_pair0_panel()
            while len(panels) < 16:
                emit_pair0_panel()

            # --- main loop over batch chunk-pairs ---
            for pair in range(N_PAIR):
                bc_lo, bc_hi = pair, pair + N_PAIR
                next_panels = {}
                c_tiles = {}
                c_done = {}

                def get_c(rb, o, side, pair=pair):
                    key = (rb + o, side)
                    if key not in c_tiles:
                        c_tiles[key] = caccpool.tile(
                            [P, B_CHUNK], F32, tag="c", name=f"c{key[0]}{side}"
                        )
                        c_done[key] = 0
                    return c_tiles[key]

                def emit_rhs(pi):
                    """Build rhs sub-tiles (4 x [P, KCG, B_CHUNK]) for product pi."""
                    rhs_spec = PRODUCTS[pi][1]
                    if rhs_spec[0] == "plain":
                        _, side, gbase = rhs_spec
                        return [panels[(side, gbase + gp)] for gp in range(4)]
                    _, op, (sa, ga), (sb, gb) = rhs_spec
                    subs = []
                    for gp in range(4):
                        sub = combopool.tile(
                            [P, KCG, B_CHUNK], BF16, tag="cb", name="cb"
                        )
                        fn = nc.vector.tensor_add if op == "add" else nc.vector.tensor_sub
                        fn(sub, panels[(sa, ga + gp)], panels[(sb, gb + gp)])
                        subs.append(sub)
                    return subs

                # DVE is strict FIFO: B-combos must be emitted ahead of the
                # previous product's C-accum drains, so each product's rhs is
                # built one product in advance (rhs for pi emitted during
                # pi-1's matmuls, before pi-1's drains).
                rhs_ready = {0: emit_rhs(0)}
                for pi, (s, rhs_spec, contribs) in enumerate(PRODUCTS):
                    subs = rhs_ready.pop(pi)

                    ps = [
                        mpsum.tile([P, B_CHUNK], F32, name=f"ps{pi}_{o}", tag="ps")
                        for o in range(2)
                    ]
                    for ic in range(KH_IC):
                        for o in range(2):
                            if isinstance(wa_t[s], list):
                                lhsT = wa_t[s][ic // 4][:, ic % 4, o * P : (o + 1) * P]
                            else:
                                lhsT = wa_t[s][:, ic, o * P : (o + 1) * P]
                            nc.tensor.matmul(
                                ps[o],
                                lhsT,
                                subs[ic // 4][:, ic % 4, :],
                                start=(ic == 0),
                                stop=(ic == KH_IC - 1),
                            )

                    # build the NEXT product's rhs before this product's
                    # drains occupy the DVE FIFO
                    if pi + 1 < len(PRODUCTS):
                        rhs_ready[pi + 1] = emit_rhs(pi + 1)

                    # C accumulation on DVE. The very last product's final
                    # drain+DMA is the only exposed tail work, so emit it in
                    # half-width pieces: the output DMA of the first half
                    # starts while the second half drains.
                    tail_split = pair == N_PAIR - 1 and pi == len(PRODUCTS) - 1
                    for rb, side, sign in contribs:
                        for o in range(2):
                            c = get_c(rb, o, side)
                            key = (rb + o, side)
                            final = c_done[key] + 1 == _C_COUNTS[(rb, side)]
                            bc = bc_lo if side == "lo" else bc_hi
                            if tail_split and final and c_done[key] > 0:
                                for h in range(2):
                                    sl = slice(h * (B_CHUNK // 2), (h + 1) * (B_CHUNK // 2))
                                    if sign > 0:
                                        nc.vector.tensor_add(c[:, sl], c[:, sl], ps[o][:, sl])
                                    else:
                                        nc.vector.tensor_sub(c[:, sl], c[:, sl], ps[o][:, sl])
                                    nc.sync.dma_start(out_d[key[0], bc, :, sl], c[:, sl])
                                c_done[key] += 1
                                continue
                            if c_done[key] == 0:
                                if sign > 0:
                                    nc.vector.tensor_copy(c, ps[o])
                                else:
                                    nc.vector.tensor_scalar_mul(c, ps[o], -1.0)
                            else:
                                if sign > 0:
                                    nc.vector.tensor_add(c, c, ps[o])
                                else:
                                    nc.vector.tensor_sub(c, c, ps[o])
                            c_done[key] += 1
                            if final:
                                nc.sync.dma_start(out_d[key[0], bc], c)

                    # spread next-pair panel prefetch across products
                    if pair + 1 < N_PAIR:
                        lo = (16 * pi) // 7
                        hi = (16 * (pi + 1)) // 7
                        for idx in range(lo, hi):
                            side, g = PANEL_ORDER[idx]
                            bc = (pair + 1) if side == "lo" else (pair + 1 + N_PAIR)
                            next_panels[(side, g)] = emit_panel(bc, g)

                panels = next_panels

    nc.compile()
    return nc


_NC_CACHE = None


def _shard_inputs(x, weight, mask):
    """Host-side marshalling: mask, Strassen A-combos, cast bf16, tile."""
    x = np.asarray(x, dtype=np.float32)
    weight = np.asarray(weight, dtype=np.float32)
    mask = np.asarray(mask, dtype=np.float32)

    # x: [8192 b, 4096 k] -> xt [16 bc, 8 g, 128 p, 4 kc, 512 b] bf16
    xT = x.T.astype(ml_dtypes.bfloat16)  # [4096, 8192]
    xt = np.ascontiguousarray(
        xT.reshape(KG, KCG, P, N_BCHUNK, B_CHUNK).transpose(3, 0, 2, 1, 4)
    )

    wm = weight * mask  # [4096 o, 4096 k]
    OH, KH = O_PER_CORE // 2, D_IN // 2
    in_maps = []
    for c in range(N_CORES):
        A = wm[c * O_PER_CORE : (c + 1) * O_PER_CORE]  # [512, 4096]
        A11, A12 = A[:OH, :KH], A[:OH, KH:]
        A21, A22 = A[OH:, :KH], A[OH:, KH:]
        combos = [A11 + A22, A21 + A22, A11, A22, A11 + A12, A21 - A11, A12 - A22]
        wa = np.empty((7, P, KH_IC, 2 * P), dtype=ml_dtypes.bfloat16)
        for i, s in enumerate(combos):
            # s [256 o, 2048 k] -> lhsT [128 p, 16 ic, 256 o], k = ic*128+p
            wa[i] = s.T.reshape(KH_IC, P, 2 * P).transpose(1, 0, 2).astype(
                ml_dtypes.bfloat16
            )
        in_maps.append({"xt": xt, "wa": wa})
    return in_maps


def kernel(x, weight, mask):
    global _NC_CACHE
    if _NC_CACHE is None:
        _NC_CACHE = build_nc()
    nc = _NC_CACHE

    in_maps = _shard_inputs(x, weight, mask)
    res = run_bass_kernel_spmd(nc, in_maps, core_ids=list(range(N_CORES)))

    out = np.empty((BATCH, D_OUT), dtype=np.float32)
    for c in range(N_CORES):
        # outT [4 ot, 16 bc, 128 p, 512 b] -> [8192 b, 512 o]
        ot = res.results[c]["outT"]
        o = ot.transpose(1, 3, 0, 2).reshape(BATCH, O_PER_CORE)
        out[:, c * O_PER_CORE : (c + 1) * O_PER_CORE] = o
    return out


# revision 20
# speedup vs baseline: 1.0190x; 1.0190x over previous
"""Trainium2 Bass kernel for ExpanderLinear: out = x @ (W * mask).T

Shapes (hardcoded): x [8192, 4096] f32, weight [4096, 4096] f32,
mask [4096, 4096] f32 -> out [8192, 4096] f32.

Strategy: tensor-parallel over output features across 8 cores, bf16,
with one level of Strassen so the PE does 7/8 of the dense matmul work.
Per core: outT_c = A @ B where A = (W*mask)[c-slice] [512, 4096] and
B = x.T [4096, 8192]. Split A 2x2 (outs 256, K 2048), B 2x2 (K 2048,
batch 4096). The 7 A-combos are built on the host (free) and shipped as
bf16 lhsT panels; the 5 B-combos are built on the idle Vector engine
from streamed x panels; products accumulate in PSUM and are assembled
into C quadrants by DVE adds, then DMA'd out.

Batch is processed as 8 chunk-pairs (bc, bc+8) of 512 columns from each
batch half. Per pair: 7 products x 2 psum tiles x 16 K-chunk matmuls
= 224 MMs (dense would be 256).

Host pre-tiles everything contiguously:
  xt [16 bc][8 g][128 p][4 kc][512 b] bf16 (panel = 512 KB)
  wa [7 s][128 p][16 ic][256 o]       bf16 (A-combo lhsT panels)
  outT [4 ot][16 bc][128 p][512 b]    f32  (host re-assembles)
"""

import ml_dtypes
import numpy as np

import concourse.bass as bass
import concourse.mybir as mybir
import concourse.tile as tile
from concourse import bacc
from concourse.bass_utils import run_bass_kernel_spmd

P = 128
D_IN = 4096
D_OUT = 4096
BATCH = 8192
N_CORES = 8
O_PER_CORE = D_OUT // N_CORES  # 512
B_CHUNK = 512
N_BCHUNK = BATCH // B_CHUNK  # 16
N_PAIR = N_BCHUNK // 2  # 8
KG = 8  # 512-row K groups (4 per K-half)
KCG = 4  # 128-row chunks per group
KH_IC = 16  # 128-row chunks per K-half
OT = 4  # 128-row output tiles per core

F32 = mybir.dt.float32
BF16 = mybir.dt.bfloat16

# Products in issue order: (wa_idx, rhs_spec, contribs)
# rhs_spec: ('plain', side, gbase) or ('combo', op, (sideA,gbaseA), (sideB,gbaseB))
# combo sub g' = panel[sideA][gbaseA+g'] op panel[sideB][gbaseB+g']
# contribs: list of (row_base, side, sign); applied to C[row_base+o'][side]
# A-combos (host order): 0:A11+A22 1:A21+A22 2:A11 3:A22 4:A11+A12
#                        5:A21-A11 6:A12-A22
PRODUCTS = [
    # M2 = (A21+A22) @ B11
    (1, ("plain", "lo", 0), [(2, "lo", +1), (2, "hi", -1)]),
    # M5 = (A11+A12) @ B22
    (4, ("plain", "hi", 4), [(0, "lo", -1), (0, "hi", +1)]),
    # M1 = (A11+A22) @ (B11+B22)
    (0, ("combo", "add", ("lo", 0), ("hi", 4)), [(0, "lo", +1), (2, "hi", +1)]),
    # M4 = A22 @ (B21-B11)
    (3, ("combo", "sub", ("lo", 4), ("lo", 0)), [(0, "lo", +1), (2, "lo", +1)]),
    # M7 = (A12-A22) @ (B21+B22)
    (6, ("combo", "add", ("lo", 4), ("hi", 4)), [(0, "lo", +1)]),
    # M3 = A11 @ (B12-B22)
    (2, ("combo", "sub", ("hi", 0), ("hi", 4)), [(0, "hi", +1), (2, "hi", +1)]),
    # M6 = (A21-A11) @ (B11+B12)
    (5, ("combo", "add", ("lo", 0), ("hi", 0)), [(2, "hi", +1)]),
]

# Total contribs per C tile (row_base, side) per pair
_C_COUNTS = {}
for _, _, contribs in PRODUCTS:
    for rb, side, _sg in contribs:
        _C_COUNTS[(rb, side)] = _C_COUNTS.get((rb, side), 0) + 1

# panel arrival order for each pair (matches product consumption order)
PANEL_ORDER = (
    [("lo", g) for g in range(4)]
    + [("hi", g) for g in range(4, 8)]
    + [("lo", g) for g in range(4, 8)]
    + [("hi", g) for g in range(4)]
)
# wa arrival order (first-needed first)
WA_ORDER = [1, 4, 0, 3, 6, 2, 5]


def build_nc():
    nc = bacc.Bacc("TRN2", target_bir_lowering=False, debug=False, num_devices=N_CORES)

    xt_d = nc.dram_tensor("xt", [N_BCHUNK, KG, P, KCG, B_CHUNK], BF16, kind="ExternalInput")
    wa_d = nc.dram_tensor("wa", [7, P, KH_IC, 2 * P], BF16, kind="ExternalInput")
    out_d = nc.dram_tensor("outT", [OT, N_BCHUNK, P, B_CHUNK], F32, kind="ExternalOutput")

    with tile.TileContext(nc) as tc:
        with (
            tc.tile_pool(name="wa", bufs=1) as wapool,
            tc.tile_pool(name="xp", bufs=18) as xppool,
            tc.tile_pool(name="combo", bufs=6) as combopool,
            tc.tile_pool(name="cacc", bufs=8) as caccpool,
            tc.tile_pool(name="mpsum", bufs=8, space="PSUM") as mpsum,
        ):
            wa_t = [None] * 7

            def emit_wa(s):
                w = wapool.tile([P, KH_IC, 2 * P], BF16, name=f"wa{s}")
                nc.sync.dma_start(w, wa_d[s])
                wa_t[s] = w

            def emit_panel(bc, g):
                xs = xppool.tile([P, KCG, B_CHUNK], BF16, tag="xs", name="xs")
                nc.sync.dma_start(xs, xt_d[bc, g])
                return xs

            # --- prologue: wa + pair-0 panels, interleaved ---
            panels = {}  # (side, g) -> tile, for current pair
            for i, s in enumerate(WA_ORDER):
                emit_wa(s)
                for _ in range(3 if i < 2 else 2):
                    if len(panels) < 16:
                        side, g = PANEL_ORDER[len(panels)]
                        bc = 0 if side == "lo" else N_PAIR
                        panels[(side, g)] = emit_panel(bc, g)
            while len(panels) < 16:
                side, g = PANEL_ORDER[len(panels)]
                bc = 0 if side == "lo" else N_PAIR
                panels[(side, g)] = emit_panel(bc, g)

            # --- main loop over batch chunk-pairs ---
            for pair in range(N_PAIR):
                bc_lo, bc_hi = pair, pair + N_PAIR
                next_panels = {}
                c_tiles = {}
                c_done = {}

                def get_c(rb, o, side, pair=pair):
                    key = (rb + o, side)
                    if key not in c_tiles:
                        c_tiles[key] = caccpool.tile(
                            [P, B_CHUNK], F32, tag="c", name=f"c{key[0]}{side}"
                        )
                        c_done[key] = 0
                    return c_tiles[key]

                def emit_rhs(pi):
                    """Build rhs sub-tiles (4 x [P, KCG, B_CHUNK]) for product pi."""
                    rhs_spec = PRODUCTS[pi][1]
                    if rhs_spec[0] == "plain":
                        _, side, gbase = rhs_spec
                        return [panels[(side, gbase + gp)] for gp in range(4)]
                    _, op, (sa, ga), (sb, gb) = rhs_spec
                    subs = []
                    for gp in range(4):
                        sub = combopool.tile(
                            [P, KCG, B_CHUNK], BF16, tag="cb", name="cb"
                        )
                        fn = nc.vector.tensor_add if op == "add" else nc.vector.tensor_sub
                        fn(sub, panels[(sa, ga + gp)], panels[(sb, gb + gp)])
                        subs.append(sub)
                    return subs

                # DVE is strict FIFO: B-combos must be emitted ahead of the
                # previous product's C-accum drains, so each product's rhs is
                # built one product in advance (rhs for pi emitted during
                # pi-1's matmuls, before pi-1's drains).
                rhs_ready = {0: emit_rhs(0)}
                for pi, (s, rhs_spec, contribs) in enumerate(PRODUCTS):
                    subs = rhs_ready.pop(pi)

                    ps = [
                        mpsum.tile([P, B_CHUNK], F32, name=f"ps{pi}_{o}", tag="ps")
                        for o in range(2)
                    ]
                    for ic in range(KH_IC):
                        for o in range(2):
                            nc.tensor.matmul(
                                ps[o],
                                wa_t[s][:, ic, o * P : (o + 1) * P],
                                subs[ic // 4][:, ic % 4, :],
                                start=(ic == 0),
                                stop=(ic == KH_IC - 1),
                            )

                    # build the NEXT product's rhs before this product's
                    # drains occupy the DVE FIFO
                    if pi + 1 < len(PRODUCTS):
                        rhs_ready[pi + 1] = emit_rhs(pi + 1)

                    # C accumulation on DVE
                    for rb, side, sign in contribs:
                        for o in range(2):
                            c = get_c(rb, o, side)
                            key = (rb + o, side)
                            if c_done[key] == 0:
                                if sign > 0:
                                    nc.vector.tensor_copy(c, ps[o])
                                else:
                                    nc.vector.tensor_scalar_mul(c, ps[o], -1.0)
                            else:
                                if sign > 0:
                                    nc.vector.tensor_add(c, c, ps[o])
                                else:
                                    nc.vector.tensor_sub(c, c, ps[o])
                            c_done[key] += 1
                            if c_done[key] == _C_COUNTS[(rb, side)]:
                                bc = bc_lo if side == "lo" else bc_hi
                                nc.sync.dma_start(out_d[key[0], bc], c)

                    # spread next-pair panel prefetch across products
                    if pair + 1 < N_PAIR:
                        lo = (16 * pi) // 7
                        hi = (16 * (pi + 1)) // 7
                        for idx in range(lo, hi):
                            side, g = PANEL_ORDER[idx]
                            bc = (pair + 1) if side == "lo" else (pair + 1 + N_PAIR)
                            next_panels[(side, g)] = emit_panel(bc, g)

                panels = next_panels

    nc.compile()
    return nc


_NC_CACHE = None


def _shard_inputs(x, weight, mask):
    """Host-side marshalling: mask, Strassen A-combos, cast bf16, tile."""
    x = np.asarray(x, dtype=np.float32)
    weight = np.asarray(weight, dtype=np.float32)
    mask = np.asarray(mask, dtype=np.float32)

    # x: [8192 b, 4096 k] -> xt [16 bc, 8 g, 128 p, 4 kc, 512 b] bf16
    xT = x.T.astype(ml_dtypes.bfloat16)  # [4096, 8192]
    xt = np.ascontiguousarray(
        xT.reshape(KG, KCG, P, N_BCHUNK, B_CHUNK).transpose(3, 0, 2, 1, 4)
    )

    wm = weight * mask  # [4096 o, 4096 k]
    OH, KH = O_PER_CORE // 2, D_IN // 2
    in_maps = []
    for c in range(N_CORES):
        A = wm[c * O_PER_CORE : (c + 1) * O_PER_CORE]  # [512, 4096]
        A11, A12 = A[:OH, :KH], A[:OH, KH:]
        A21, A22 = A[OH:, :KH], A[OH:, KH:]
        combos = [A11 + A22, A21 + A22, A11, A22, A11 + A12, A21 - A11, A12 - A22]
        wa = np.empty((7, P, KH_IC, 2 * P), dtype=ml_dtypes.bfloat16)
        for i, s in enumerate(combos):
            # s [256 o, 2048 k] -> lhsT [128 p, 16 ic, 256 o], k = ic*128+p
            wa[i] = s.T.reshape(KH_IC, P, 2 * P).transpose(1, 0, 2).astype(
                ml_dtypes.bfloat16
            )
        in_maps.append({"xt": xt, "wa": wa})
    return in_maps


def kernel(x, weight, mask):
    global _NC_CACHE
    if _NC_CACHE is None:
        _NC_CACHE = build_nc()
    nc = _NC_CACHE

    in_maps = _shard_inputs(x, weight, mask)
    res = run_bass_kernel_spmd(nc, in_maps, core_ids=list(range(N_CORES)))

    out = np.empty((BATCH, D_OUT), dtype=np.float32)
    for c in range(N_CORES):
        # outT [4 ot, 16 bc, 128 p, 512 b] -> [8192 b, 512 o]
        ot = res.results[c]["outT"]
        o = ot.transpose(1, 3, 0, 2).reshape(BATCH, O_PER_CORE)
        out[:, c * O_PER_CORE : (c + 1) * O_PER_CORE] = o
    return out
